# revision 33
# baseline (speedup 1.0000x reference)
"""MoE layer (N=4096, D=1024, H=4096, E=8, top-2) on 8 Trainium2 cores.

Strategy: hidden-dim tensor-parallel (replaces expert-parallel).
  - Host computes the small gate, top-2 ids and softmax weights, groups the
    8192 (token, expert) pairs by expert, and replicates the grouped
    activations to all 8 cores.
  - Core c holds the hidden slice [c*512, (c+1)*512) of ALL experts' W1/W2
    (SBUF-resident, loaded once) and computes for every pair the partial
    FFN over its slice:
        h = relu(x @ W1[e][:, sl] + b1[e][sl]);  yT_partial = W2-contract
    Every core runs the exact same 8192 pair columns (per-expert group
    sizes baked in at compile time), so there is zero capacity padding and
    perfect load balance regardless of routing skew.
  - Host sums the 8 partial outputs, adds b2 and the gate-weighted scatter
    into the [N, D] output.

Device kernel (identical SPMD program on all 8 cores):
  - All matmul operands fp16 (bf16 PE rate, fp32 PSUM accumulation);
    halves SBUF/HBM traffic vs fp32r, no >=256 free-dim rate rule, and the
    lower data-movement power avoids the P0 PE down-clock seen with f32r.
  - Pairs processed in expert-pure chunks of <=512 columns:
      gemm1: h[hm][:, chunk]  = relu(sum_dk W1t.T @ x)     (ACT drains PSUM)
      gemm2: yT[dc][:, chunk] = sum_hk W2t.T @ h           (DVE drains PSUM)
  - DMA rings are packet-rate limited (~128 partition-line packets per
    descriptor, ~90-100 packets/us; fat lines run at ~280GB/s), so ALL
    streams are host-packed into one-descriptor fat-line blocks:
    weights [128, 4096] per expert per matrix, x [128, DK*group_width] per
    chunk-group, y [128, DCT*chunk_width] per chunk.
  - Ring plan: sync carries x groups + weights interleaved just-in-time;
    gpsimd/scalar alternate the per-chunk y writes (descriptor slots on
    the ACT ring stay shallow so its relus are never blocked); the last
    two chunks are small and split their y across both rings to shrink
    the post-compute drain tail.
"""

import numpy as np

from concourse import bacc
import concourse.mybir as mybir
from concourse.tile import TileContext
import concourse.bass_utils as bass_utils

N_TOK, D, H, E, TOPK = 4096, 1024, 4096, 8, 2
NCORES = 8
PAIRS = N_TOK * TOPK  # 8192 (token, expert) pairs, expert-grouped
HS = H // NCORES      # 512 hidden units per core
DK = D // 128         # 8 contraction tiles for gemm1
HMT = HS // 128       # 4 hidden tiles (gemm1 out / gemm2 contraction)
DCT = D // 128        # 8 output-column tiles for gemm2
CHUNK = 512           # max pair-chunk width (one fp32 PSUM bank)
XGRP = 3              # chunks per packed x descriptor (steady state)
WARM_MM = 24          # HAM warmup matmuls: must span until the first x/W
                      # descriptors land (~16us) — a PE idle gap >3.4us
                      # here would re-throttle the clock gate (HAM MID)
TAIL_W = 192          # width of the final drain-friendly chunk

TRACE = False
TRACE_CORES = None
LAST_RESULTS = None

_NC_CACHE = {}


def _chunks(counts):
    """Expert-pure, balanced pair chunks of width <= CHUNK.

    The last chunk is split small (TAIL_W) so the final y drain after the
    last matmul is short.
    """
    out = []
    off = 0
    for e, n in enumerate(counts):
        k = -(-n // CHUNK) if n else 0
        for i in range(k):
            w = n // k + (1 if i < n % k else 0)
            out.append([e, off, w])
            off += w
    if out and out[-1][2] > TAIL_W + 64:
        e, off, w = out[-1]
        out[-1] = [e, off, w - TAIL_W]
        out.append([e, off + w - TAIL_W, TAIL_W])
    return [tuple(c) for c in out]


def _xgroups(chunks):
    """Chunk groups for packed x descriptors: [c0], [c1,c2], then XGRP."""
    groups = [(0, 1)]
    if len(chunks) > 2:
        groups.append((1, 2))
    i = 3
    while i < len(chunks):
        n = min(XGRP, len(chunks) - i)
        groups.append((i, n))
        i += n
    return groups


def _build_nc(counts):
    f16, f32 = mybir.dt.float16, mybir.dt.float32
    Relu = mybir.ActivationFunctionType.Relu
    nc = bacc.Bacc("TRN2", target_bir_lowering=False)
    xP = nc.dram_tensor("xP", [128, DK * PAIRS], f16, kind="ExternalInput")
    W1 = nc.dram_tensor("W1", [E * 128, DK * HS], f16, kind="ExternalInput")
    W2 = nc.dram_tensor("W2", [E * 128, HMT * D], f16, kind="ExternalInput")
    b1 = nc.dram_tensor("b1", [128, E * HMT], f32, kind="ExternalInput")
    yP = nc.dram_tensor("yP", [128, DCT * PAIRS], f16, kind="ExternalOutput")

    chunks = _chunks(counts)
    groups = _xgroups(chunks)
    n_chunks = len(chunks)
    xgrp_max = max(sum(chunks[c0 + j][2] for j in range(ng)) for c0, ng in groups)

    with TileContext(nc) as tc:
        with (
            tc.tile_pool(name="w1p", bufs=1) as w1p,
            tc.tile_pool(name="w2p", bufs=1) as w2p,
            tc.tile_pool(name="xp", bufs=2) as xp,
            tc.tile_pool(name="hp", bufs=1) as hp,
            tc.tile_pool(name="yp", bufs=2) as yp,
            tc.tile_pool(name="cp", bufs=1) as cp,
            tc.tile_pool(name="ps1", bufs=3, space="PSUM") as ps1,
            tc.tile_pool(name="ps2", bufs=3, space="PSUM") as ps2,
        ):
            # --- HAM warmup: dummy matmuls keep the PE busy (and release
            # the clock gate) until the first weight/x descriptors land ---
            warm = cp.tile([128, CHUNK], f16, tag="warm", name="warm")
            nc.vector.memset(warm, 0.0)
            wps = ps2.tile([128, CHUNK], f32, tag="ps2", name="warmps")
            for i in range(WARM_MM):
                nc.tensor.matmul(
                    wps, warm[:, :128], warm, start=(i == 0), stop=(i == WARM_MM - 1)
                )

            b1t = cp.tile([128, E * HMT], f32, tag="b1", name="b1t")
            nc.gpsimd.dma_start(out=b1t, in_=b1[:, :])

            # --- sync ring: x groups + weights interleaved so everything
            # arrives 1-2 chunk-groups ahead of its first use ---
            w1t = [None] * E
            w2t = [None] * E

            def load_w(e, which, eng=None):
                eng = eng or nc.sync
                if which == 1:
                    t = w1p.tile([128, DK * HS], f16, tag=f"w1_{e}", name=f"w1t{e}")
                    eng.dma_start(out=t, in_=W1[e * 128 : (e + 1) * 128, :])
                    w1t[e] = t
                else:
                    t = w2p.tile([128, HMT * D], f16, tag=f"w2_{e}", name=f"w2t{e}")
                    eng.dma_start(out=t, in_=W2[e * 128 : (e + 1) * 128, :])
                    w2t[e] = t

            xtiles = [None] * n_chunks
            # startup critical path: W1[e0] leads the sync ring while the
            # first x group leads the scalar ring (which spins up ~2us
            # later); two fat descriptors on the ACT ring don't back it up
            # enough to delay the relus behind them
            load_w(0, 1)
            next_e = 1
            for gi, (c0, ng) in enumerate(groups):
                off = chunks[c0][1]
                gw = sum(chunks[c0 + j][2] for j in range(ng))
                gt = xp.tile([128, DK * xgrp_max], f16, tag="xg", name=f"xg{gi}")
                xeng = nc.scalar if gi == 0 else nc.sync
                xeng.dma_start(
                    out=gt[:, : DK * gw], in_=xP[:, DK * off : DK * (off + gw)]
                )
                sub = 0
                for j in range(ng):
                    w = chunks[c0 + j][2]
                    xtiles[c0 + j] = [
                        gt[:, dk * gw + sub : dk * gw + sub + w] for dk in range(DK)
                    ]
                    sub += w
                # weight stream rides along: W2[e0] after group 0, then one
                # expert's pair per group — always far ahead of its chunks
                if gi == 0:
                    load_w(0, 2, nc.scalar)
                elif next_e < E:
                    load_w(next_e, 1)
                    load_w(next_e, 2)
                    next_e += 1
            while next_e < E:
                load_w(next_e, 1)
                load_w(next_e, 2)
                next_e += 1

            for ci, (e, off, w) in enumerate(chunks):
                xt = xtiles[ci]
                # gemm1: h[hm] = relu(sum_dk W1.T @ x + b1)
                ht = [
                    hp.tile([128, CHUNK], f16, tag=f"h{hm}", name=f"ht{hm}")
                    for hm in range(HMT)
                ]
                for hm in range(HMT):
                    ps = ps1.tile([128, CHUNK], f32, tag="ps1", name="ps1t")
                    for dk in range(DK):
                        nc.tensor.matmul(
                            ps[:, :w],
                            w1t[e][:, dk * HS + hm * 128 : dk * HS + (hm + 1) * 128],
                            xt[dk],
                            start=(dk == 0),
                            stop=(dk == DK - 1),
                        )
                    col = e * HMT + hm
                    nc.scalar.activation(
                        ht[hm][:, :w], ps[:, :w], Relu, bias=b1t[:, col : col + 1]
                    )
                # gemm2: yT[dc] = sum_hk W2.T @ h, packed into one y tile
                yt = yp.tile([128, DCT * CHUNK], f16, tag="yt", name="yt")
                for dc in range(DCT):
                    ps = ps2.tile([128, CHUNK], f32, tag="ps2", name="ps2t")
                    for hk in range(HMT):
                        nc.tensor.matmul(
                            ps[:, :w],
                            w2t[e][:, hk * D + dc * 128 : hk * D + (dc + 1) * 128],
                            ht[hk][:, :w],
                            start=(hk == 0),
                            stop=(hk == HMT - 1),
                        )
                    nc.vector.tensor_copy(yt[:, dc * w : (dc + 1) * w], ps[:, :w])
                if ci >= n_chunks - 2:
                    # drain tail: split across both rings
                    half = DCT // 2 * w
                    nc.gpsimd.dma_start(
                        out=yP[:, DCT * off : DCT * off + half], in_=yt[:, :half]
                    )
                    nc.scalar.dma_start(
                        out=yP[:, DCT * off + half : DCT * (off + w)],
                        in_=yt[:, half : DCT * w],
                    )
                else:
                    oeng = nc.gpsimd if ci % 2 == 0 else nc.scalar
                    oeng.dma_start(
                        out=yP[:, DCT * off : DCT * (off + w)], in_=yt[:, : DCT * w]
                    )
    nc.compile()
    return nc


def _get_nc(counts):
    if counts not in _NC_CACHE:
        _NC_CACHE[counts] = _build_nc(counts)
    return _NC_CACHE[counts]


def kernel(x, Wg, bg, W1, b1, W2, b2):
    global LAST_RESULTS
    x = np.asarray(x, dtype=np.float32)
    Wg = np.asarray(Wg, dtype=np.float32)
    bg = np.asarray(bg, dtype=np.float32)
    W1 = np.asarray(W1, dtype=np.float32)
    b1 = np.asarray(b1, dtype=np.float32)
    W2 = np.asarray(W2, dtype=np.float32)
    b2 = np.asarray(b2, dtype=np.float32)

    # --- gate + top-k routing (replicated small gate, on host) ---
    g = x @ Wg + bg  # [N, E]
    order = np.argsort(-g, axis=1, kind="stable")[:, :TOPK]  # [N, 2]
    topv = np.take_along_axis(g, order, axis=1)
    topv = topv - topv.max(axis=1, keepdims=True)
    ex = np.exp(topv)
    sw = ex / ex.sum(axis=1, keepdims=True)  # [N, 2] softmax over selected

    counts = tuple(int((order == e).sum()) for e in range(E))
    nc = _get_nc(counts)
    chunks = _chunks(counts)
    groups = _xgroups(chunks)

    # --- dispatch: expert-grouped pair order, replicated to all cores ---
    pos = np.empty((N_TOK, TOPK), np.int64)  # (token, k) -> pair column
    offs = []
    toks = []
    off = 0
    for e in range(E):
        tok, kk = np.where(order == e)
        pos[tok, kk] = off + np.arange(tok.size)
        offs.append(off)
        toks.append(tok)
        off += tok.size
    tok_all = np.concatenate(toks)
    xT = x[tok_all].T.astype(np.float16)  # [D, PAIRS]

    # pack x per chunk-group: [128, DK*gw] blocks, dk-major columns
    xPk = np.empty((128, DK * PAIRS), np.float16)
    for c0, ng in groups:
        o = chunks[c0][1]
        gw = sum(chunks[c0 + j][2] for j in range(ng))
        blk = xT[:, o : o + gw].reshape(DK, 128, gw).transpose(1, 0, 2)
        xPk[:, DK * o : DK * (o + gw)] = blk.reshape(128, DK * gw)

    in_maps = []
    for c in range(NCORES):
        sl = slice(c * HS, (c + 1) * HS)
        # pack each expert's weight slice as one [128, DK*HS] / [128, HMT*D]
        # row-block so it loads as a single fat-lined DMA descriptor
        W1s = np.ascontiguousarray(
            W1[:, :, sl]
            .reshape(E, DK, 128, HS)
            .transpose(0, 2, 1, 3)
            .reshape(E * 128, DK * HS)
        ).astype(np.float16)
        W2s = np.ascontiguousarray(
            W2[:, sl, :]
            .reshape(E, HMT, 128, D)
            .transpose(0, 2, 1, 3)
            .reshape(E * 128, HMT * D)
        ).astype(np.float16)
        b1s = np.ascontiguousarray(
            b1[:, sl].reshape(E, HMT, 128).transpose(2, 0, 1).reshape(128, E * HMT)
        )
        in_maps.append({"xP": xPk, "W1": W1s, "W2": W2s, "b1": b1s})

    kwargs = {}
    if TRACE_CORES is not None:
        kwargs["trace_cores"] = TRACE_CORES
    LAST_RESULTS = bass_utils.run_bass_kernel_spmd(
        nc, in_maps, core_ids=list(range(NCORES)), trace=TRACE, **kwargs
    )

    # --- combine: sum partials over cores, unpack, add b2, gate-weighted
    # scatter into the final [N, D] output ---
    Ps = np.zeros((128, DCT * PAIRS), np.float32)
    for r in LAST_RESULTS.results:
        Ps += r["yP"].astype(np.float32)
    Y = np.empty((PAIRS, D), np.float32)  # pair-major
    for e, off, w in chunks:
        blk = Ps[:, DCT * off : DCT * (off + w)].reshape(128, DCT, w)
        Y[off : off + w] = blk.transpose(1, 0, 2).reshape(D, w).T
    for e in range(E):
        if np.any(b2[e]):
            Y[offs[e] : offs[e] + counts[e]] += b2[e][None, :]
    out = sw[:, 0, None] * Y[pos[:, 0]] + sw[:, 1, None] * Y[pos[:, 1]]
    return out.astype(np.float32)
